# revision 1
# baseline (speedup 1.0000x reference)
"""Trainium2 Bass kernel for nn_MultiHeadAttention_9878424780806.

Problem (hardcoded): B=4, S=2048, D=1024, H=16 heads (head_dim 64), fp32.
  y = softmax((x@Wq)(x@Wk)^T / 8 + mask) @ (x@Wv) @ Wout   (+ zero biases)

Sharding: 8 cores = 4 batches x 2 head-halves (8 heads per core).
Each core computes a partial y for its batch from its 8 heads; the host
sums the two partials per batch (the out-projection is linear in heads).
The host feeds x pre-transposed per batch (xT [D, S]) - a data-layout
choice that avoids 128 on-chip 128x128 transposes per core.

Per-core structure (one fused, software-pipelined phase):
  - load xT [d, t] in 4 independent t-chunks; QK projection of pair 0
  - V = x @ Wv_half in natural [t,f] layout, stored per k-tile and per head
    with a ones column appended (the ones column makes the attn@V matmul
    emit softmax row-sums as an extra output row for free); attention
    consumes v1 per k-tile, so it starts while V-proj is still running
  - per head-pair j: per head, per 1024-wide q chunk:
      scoresT[k,q] per k-tile -> exp on ScalarE (scale=1/8 folded in) ->
      U[0:65,q] += [V|1]^T-matmul accumulated over k in PSUM; U is copied
      to SBUF immediately (frees the PSUM accumulator), then normalized:
      r = 1/U[64] broadcast across partitions via a DRAM bounce;
      ao = U[0:64]*r lands in ao_pair [128 f, 2048 t] (head 1 crosses
      partitions 0:64 -> 64:128 via an SBUF-to-SBUF DMA);
    pair j+1's QK projection is emitted mid-pair so it fills PE gaps;
    out-projection per q-half: y += ao_pair^T @ Wout_pair via DMA
    accumulate (CCE add) into the pre-zeroed y buffer, t-order rotated
    per pair so pairs don't serialize on y regions.

All matmuls run as float32r (TF32-like, 1 col/cycle at N>=256); plain fp32
matmul on TRN2 is 4x slower. attn_mask and the biases are all-zero by
construction (spec fill=zeros); kernel() refuses non-zero values.
"""

import numpy as np
from contextlib import ExitStack

import concourse.bass as bass
import concourse.tile as tile
from concourse import bacc, mybir
from concourse.bass_utils import run_bass_kernel_spmd

F32 = mybir.dt.float32
F32R = mybir.dt.float32r

B, S, D, H = 4, 2048, 1024, 16
HD = 64          # head dim
HPC = 8          # heads per core
N_CORES = 8

TT = S // 128    # 16 token tiles
DT = D // 128    # 8 d tiles
FH = HPC * HD    # 512 features per core half
NPAIR = HPC // 2


def build_program():
    nc = bacc.Bacc("TRN2", target_bir_lowering=False, debug=False,
                   enable_asserts=True, num_devices=N_CORES)

    xt_ap = nc.dram_tensor("xt", [D, S], F32R, kind="ExternalInput").ap()
    wq_ap = nc.dram_tensor("wq", [D, FH], F32R, kind="ExternalInput").ap()
    wk_ap = nc.dram_tensor("wk", [D, FH], F32R, kind="ExternalInput").ap()
    wv_ap = nc.dram_tensor("wv", [D, FH], F32R, kind="ExternalInput").ap()
    wout_ap = nc.dram_tensor("wout", [FH, D], F32R, kind="ExternalInput").ap()
    y_ap = nc.dram_tensor("y", [S, D], F32, kind="ExternalOutput").ap()

    xt_r = xt_ap.rearrange("(dt p) t -> p dt t", p=128)
    wq_r = wq_ap.rearrange("(dt p) f -> p dt f", p=128)
    wk_r = wk_ap.rearrange("(dt p) f -> p dt f", p=128)
    wv_r = wv_ap.rearrange("(dt p) f -> p dt f", p=128)
    wout_r = wout_ap.rearrange("(ft p) e -> p ft e", p=128)

    with tile.TileContext(nc) as tc, ExitStack() as ctx:
        xtp = ctx.enter_context(tc.tile_pool(name="xTp", bufs=1))
        v1p = ctx.enter_context(tc.tile_pool(name="v1p", bufs=1))
        psp = ctx.enter_context(tc.tile_pool(name="psp", bufs=2, space="PSUM"))
        pup = ctx.enter_context(tc.tile_pool(name="pup", bufs=1, space="PSUM"))
        psyp = ctx.enter_context(tc.tile_pool(name="psyp", bufs=1, space="PSUM"))
        rdp = ctx.enter_context(tc.tile_pool(name="rdp", bufs=2, space="DRAM"))
        wqkp = ctx.enter_context(tc.tile_pool(name="wqkp", bufs=1))
        qkp = ctx.enter_context(tc.tile_pool(name="qkp", bufs=2))
        wop = ctx.enter_context(tc.tile_pool(name="woutp", bufs=1))
        attnp = ctx.enter_context(tc.tile_pool(name="attnp", bufs=6))
        unp = ctx.enter_context(tc.tile_pool(name="unp", bufs=2))
        rbp = ctx.enter_context(tc.tile_pool(name="rbp", bufs=1))
        aopp = ctx.enter_context(tc.tile_pool(name="aopairp", bufs=2))
        ao1p = ctx.enter_context(tc.tile_pool(name="ao1p", bufs=1))
        yp = ctx.enter_context(tc.tile_pool(name="youtp", bufs=2))

        # ---- loads: W slices (HWDGE) + xT chunks (SWDGE), in parallel ----
        def emit_proj_load(j):
            wqk = wqkp.tile([128, DT, 256], F32R, tag="wqk")
            nc.sync.dma_start(wqk[:, :, 0:128],
                              wq_r[:, :, j * 128:(j + 1) * 128])
            nc.sync.dma_start(wqk[:, :, 128:256],
                              wk_r[:, :, j * 128:(j + 1) * 128])
            return wqk

        wqk0 = emit_proj_load(0)

        # xTc[c][p, dt, t'] = x[c*512+t', dt*128+p]
        xTc = []
        for c in range(4):
            t = xtp.tile([128, DT, 512], F32R, tag=f"xt{c}")
            eng = nc.gpsimd if c % 2 else nc.sync
            eng.dma_start(t[:, :, :], xt_r[:, :, c * 512:(c + 1) * 512])
            xTc.append(t)

        # wv staging borrows a qkT-sized slot (same 16KB/partition shape)
        wv_t = qkp.tile([128, DT, FH], F32R, tag="qkT")
        nc.sync.dma_start(wv_t[:, :, :], wv_r[:, :, :])

        def emit_proj_mm(wqk):
            """QK projection -> qkT tile [128, {Q,K}, S]."""
            qkT = qkp.tile([128, 2, S], F32R, tag="qkT")
            for fs in range(2):
                for tck in range(4):
                    psq = psyp.tile([128, 512], F32, tag="psy")
                    for dt in range(DT):
                        nc.tensor.matmul(
                            psq[:, :],
                            wqk[:, dt, fs * 128:(fs + 1) * 128],
                            xTc[tck][:, dt, :],
                            start=(dt == 0), stop=(dt == DT - 1))
                    nc.vector.tensor_copy(
                        qkT[:, fs, tck * 512:(tck + 1) * 512], psq[:, :])
            return qkT

        qkT = emit_proj_mm(wqk0)
        UPFRONT_VPROJ = False

        # ---- V projection (natural layout, all heads), per k-tile tiles ----
        # v1k[kt][p, h, 0:64] = V[kt*128+p, h*64+c]; v1k[kt][p, h, 64] = 1.0
        # Units are emitted lazily (interleaved into the first attention
        # chunk) so the ScalarE exp pipeline starts as early as possible.
        v1k = [None] * TT

        def emit_vproj(kt):
            if v1k[kt] is not None:
                return
            v1 = v1p.tile([128, HPC, HD + 1], F32R, tag=f"v1_{kt}")
            nc.vector.memset(v1[:, :, HD:HD + 1].bitcast(F32), 1.0)
            xc, sub = xTc[kt // 4], kt % 4
            psv = psyp.tile([128, 512], F32, tag="psy")
            for dt in range(DT):
                nc.tensor.matmul(psv[:, :],
                                 xc[:, dt, sub * 128:(sub + 1) * 128],
                                 wv_t[:, dt, :],
                                 start=(dt == 0), stop=(dt == DT - 1))
            nc.vector.tensor_copy(
                v1[:, :, 0:HD],
                psv[:, :].rearrange("p (h c) -> p h c", c=HD))
            v1k[kt] = v1

        if UPFRONT_VPROJ:
            for kt in range(TT):
                emit_vproj(kt)

        def emit_normalize(hs, q0, pu, ao_pair):
            # move U out of PSUM right away, then normalize from SBUF:
            # r = 1/rowsum broadcast across partitions via a DRAM bounce
            # (SBUF APs can't be 0-stride on the partition dim)
            u = unp.tile([HD + 1, 1024], F32, tag="u")
            nc.vector.tensor_copy(u[:, :], pu[0:HD + 1, :])
            rb = rbp.tile([HD, 1024], F32, tag="rb")
            nc.vector.reciprocal(rb[0:1, :], u[HD:HD + 1, :])
            rd = rdp.tile([1, 1024], F32, tag="rd")
            nc.sync.dma_start(rd[:, :], rb[0:1, :])
            nc.sync.dma_start(rb[:, :], rd[0:1, :].to_broadcast((HD, 1024)))
            if hs == 0:
                nc.vector.tensor_mul(ao_pair[0:HD, q0:q0 + 1024],
                                     u[0:HD, :], rb[:, :])
            else:
                # head 1's U sits on partitions 0:64 but belongs at rows
                # 64:128 of ao_pair; cross partitions via SBUF->SBUF DMA
                ao1 = ao1p.tile([HD, 1024], F32R, tag="ao1")
                nc.vector.tensor_mul(ao1[:, :], u[0:HD, :], rb[:, :])
                nc.sync.dma_start(
                    ao_pair[HD:2 * HD, q0:q0 + 1024], ao1[:, :])

        def emit_attention(j, hs, qh, qkT, ao_pair):
            """One head's attention for one 1024-wide q chunk."""
            h8 = j * 2 + hs
            lo, hi = hs * HD, (hs + 1) * HD
            q0 = qh * 1024
            pu = pup.tile([HD + 1, 1024], F32, tag="pu")
            for kt in range(TT):
                # first pass: keep V-proj 4 k-tiles ahead of attnV so exp
                # starts early but attnV never waits on V (no-op later)
                for pf in range(min(kt + 5, TT)):
                    emit_vproj(pf)
                ps = psp.tile([128, 1024], F32, tag="ps")
                for qc in range(2):
                    nc.tensor.matmul(
                        ps[:, qc * 512:(qc + 1) * 512],
                        qkT[lo:hi, 1, kt * 128:(kt + 1) * 128],
                        qkT[lo:hi, 0, q0 + qc * 512:q0 + (qc + 1) * 512],
                        start=True, stop=True)
                at = attnp.tile([128, 1024], F32R, tag="attn")
                nc.scalar.activation(
                    at[:, :], ps[:, :],
                    func=mybir.ActivationFunctionType.Exp,
                    scale=0.125)
                for qc in range(2):
                    nc.tensor.matmul(
                        pu[0:HD + 1, qc * 512:(qc + 1) * 512],
                        v1k[kt][:, h8, :],
                        at[:, qc * 512:(qc + 1) * 512],
                        start=(kt == 0), stop=(kt == TT - 1))
            emit_normalize(hs, q0, pu, ao_pair)

        def emit_outproj(j, qh, ao_pair, wout_t):
            for i in range(8):
                tt = qh * 8 + (i + j * 2) % 8  # rotate per pair
                psy = psyp.tile([128, 1024], F32, tag="psy")
                ysb = yp.tile([128, D], F32, tag="y")
                for ec in range(2):
                    nc.tensor.matmul(
                        psy[:, ec * 512:(ec + 1) * 512],
                        ao_pair[:, tt * 128:(tt + 1) * 128],
                        wout_t[:, 0, ec * 512:(ec + 1) * 512],
                        start=True, stop=True)
                    nc.vector.tensor_copy(ysb[:, ec * 512:(ec + 1) * 512],
                                          psy[:, ec * 512:(ec + 1) * 512])
                nc.gpsimd.dma_start(y_ap[tt * 128:(tt + 1) * 128, :],
                                    ysb[:, :],
                                    accum_op=mybir.AluOpType.add)

        for j in range(NPAIR):
            wout_t = wop.tile([128, 1, D], F32R, tag="wout")
            nc.sync.dma_start(wout_t[:, :, :], wout_r[:, j:j + 1, :])
            ao_pair = aopp.tile([128, S], F32R, tag="aopair")
            emit_attention(j, 0, 0, qkT, ao_pair)
            if j + 1 < NPAIR:
                wqk_next = emit_proj_load(j + 1)
            emit_attention(j, 1, 0, qkT, ao_pair)
            if j + 1 < NPAIR:
                qkT_next = emit_proj_mm(wqk_next)
            else:
                qkT_next = None
            emit_outproj(j, 0, ao_pair, wout_t)
            emit_attention(j, 0, 1, qkT, ao_pair)
            emit_attention(j, 1, 1, qkT, ao_pair)
            emit_outproj(j, 1, ao_pair, wout_t)
            qkT = qkT_next

    nc.compile()
    return nc


_NC = None


def get_nc():
    global _NC
    if _NC is None:
        _NC = build_program()
    return _NC


def make_in_maps(x, Wqkv, Wout):
    x = np.asarray(x, dtype=np.float32)
    Wqkv = np.asarray(Wqkv, dtype=np.float32)
    Wout = np.asarray(Wout, dtype=np.float32)
    in_maps = []
    for b in range(B):
        xbt = np.ascontiguousarray(x[b].T)
        for hh in range(2):
            c0 = hh * FH
            in_maps.append({
                "xt": xbt,
                "wq": np.ascontiguousarray(Wqkv[:, c0:c0 + FH]),
                "wk": np.ascontiguousarray(Wqkv[:, D + c0:D + c0 + FH]),
                "wv": np.ascontiguousarray(Wqkv[:, 2 * D + c0:2 * D + c0 + FH]),
                "wout": np.ascontiguousarray(Wout[c0:c0 + FH, :]),
            })
    return in_maps


def assemble(results):
    y = np.empty((B, S, D), dtype=np.float32)
    for b in range(B):
        y[b] = results[2 * b]["y"] + results[2 * b + 1]["y"]
    return y


def kernel(x, attn_mask, Wqkv, bqkv, Wout, bout):
    for name, t in (("attn_mask", attn_mask), ("bqkv", bqkv), ("bout", bout)):
        if np.any(np.asarray(t)):
            raise NotImplementedError(f"kernel assumes {name} == 0")
    nc = get_nc()
    res = run_bass_kernel_spmd(nc, make_in_maps(x, Wqkv, Wout),
                               core_ids=list(range(N_CORES)))
    return assemble(res.results)


if __name__ == "__main__":
    rng = np.random.default_rng(0)
    x = rng.standard_normal((B, S, D), dtype=np.float32)
    Wqkv = (rng.standard_normal((D, 3 * D), dtype=np.float32) / np.sqrt(D)).astype(np.float32)
    Wout = (rng.standard_normal((D, D), dtype=np.float32) / np.sqrt(D)).astype(np.float32)
    zeros = np.zeros
    y = kernel(x, zeros((S, S), np.float32), Wqkv, zeros(3 * D, np.float32),
               Wout, zeros(D, np.float32))
    print("y", y.shape, y.dtype, float(np.abs(y).mean()))



# revision 75
# speedup vs baseline: 1.3746x; 1.3746x over previous
"""Trainium2 Bass kernel for nn_MultiHeadAttention_9878424780806.

Problem (hardcoded): B=4, S=2048, D=1024, H=16 heads (head_dim 64), fp32.
  y = softmax((x@Wq)(x@Wk)^T / 8 + mask) @ (x@Wv) @ Wout   (+ zero biases)

Sharding: 8 cores = 4 batches x 2 head-halves (8 heads per core); the host
sums the two partial y's per batch (out-projection is linear in heads) and
feeds x pre-transposed per batch (xT [D, S]).

Inputs are fed as bf16 (host-converted): every matmul contraction here is
either softmax-bound (score noise ~2e-3 logit units -> ~0.2% final) or a
plain dense layer (bf16 rounding ~0.1%), comfortably inside the 2e-2 gate,
and bf16 stationaries enable Fast Weight Load (2x LDWEIGHTS) plus halve
DMA/SBUF traffic. PSUM accumulation stays fp32.

Schedule (the point of this revision): the PE has ~786k columns of matmul
work (scores 262k + attnV 262k + projections 262k) and the ScalarE has
~267us of exp work. Inside an attention chunk the per-k-tile chain
scores->exp->attnV makes ScalarE the pacing engine (exp ~1030ns vs 852ns
of PE per k-tile), so all non-attention matmuls (next pair's QK
projection, the out-projection) are diced into ~213ns filler units
consumed one-or-two per k-tile from a global deque: the PE never idles
waiting on exp. Scores are emitted one k-tile ahead of attnV to keep the
exp latency off the PE critical path. Startup loads ride the two HWDGE
queues critical-first (queue order is priority), y accumulates on-chip
across pairs (one 8MB write instead of 32MB of DMA-accumulate), and the
softmax reciprocal is broadcast across partitions on GpSimd instead of a
DRAM bounce.
"""

import numpy as np
from collections import deque
from contextlib import ExitStack

import concourse.bass as bass
import concourse.tile as tile
from concourse import bacc, mybir
from concourse.bass_utils import run_bass_kernel_spmd

F32 = mybir.dt.float32
BF16 = mybir.dt.bfloat16

B, S, D, H = 4, 2048, 1024, 16
HD = 64          # head dim
HPC = 8          # heads per core
N_CORES = 8

TT = S // 128    # 16 token tiles
DT = D // 128    # 8 d tiles
FH = HPC * HD    # 512 features per core half
NPAIR = HPC // 2

# (fs, tck) order for QK projection chains: both fs of a tck before the
# next tck, so the first chains only need the earliest xT chunks.
PROJ_ORDER = [(fs, tck) for tck in range(4) for fs in range(2)]

import os
RATE = [float(x) for x in os.environ.get(
    "FILL_RATE", "2.0,1.5,1.25,1.0").split(",")]
WARMUP = int(os.environ.get("WARMUP", "10"))


def build_program():
    nc = bacc.Bacc("TRN2", target_bir_lowering=False, debug=False,
                   enable_asserts=True, num_devices=N_CORES)

    # all inputs arrive pre-packed in tile layout (host-side transpose) so
    # every load is a single contiguous block, not 1024 small descriptors
    xt_r = nc.dram_tensor("xt", [4, 128, DT, 512], BF16,
                          kind="ExternalInput").ap()
    wq_r = nc.dram_tensor("wq", [NPAIR, 128, DT, 128], BF16,
                          kind="ExternalInput").ap()
    wk_r = nc.dram_tensor("wk", [NPAIR, 128, DT, 128], BF16,
                          kind="ExternalInput").ap()
    wv_r = nc.dram_tensor("wv", [128, DT, FH], BF16,
                          kind="ExternalInput").ap()
    wout_r = nc.dram_tensor("wout", [NPAIR, 128, D], BF16,
                            kind="ExternalInput").ap()
    y_ap = nc.dram_tensor("y", [S, D], F32, kind="ExternalOutput").ap()

    with tile.TileContext(nc) as tc, ExitStack() as ctx:
        xtp = ctx.enter_context(tc.tile_pool(name="xTp", bufs=1))
        v1p = ctx.enter_context(tc.tile_pool(name="v1p", bufs=1))
        psp = ctx.enter_context(tc.tile_pool(name="psp", bufs=2, space="PSUM"))
        pup = ctx.enter_context(tc.tile_pool(name="pup", bufs=1, space="PSUM"))
        # proj/vproj accumulator chains + outproj tiles share a 2-slot ring
        pcp = ctx.enter_context(tc.tile_pool(name="pcp", bufs=2, space="PSUM"))
        wqkp = ctx.enter_context(tc.tile_pool(name="wqkp", bufs=1))
        qkp = ctx.enter_context(tc.tile_pool(name="qkp", bufs=3))
        wop = ctx.enter_context(tc.tile_pool(name="woutp", bufs=2))
        attnp = ctx.enter_context(tc.tile_pool(name="attnp", bufs=int(os.environ.get("ATTNP","6"))))
        unp = ctx.enter_context(tc.tile_pool(name="unp", bufs=2))
        rbp = ctx.enter_context(tc.tile_pool(name="rbp", bufs=2))
        ao1p = ctx.enter_context(tc.tile_pool(name="ao1p", bufs=2))
        aopp = ctx.enter_context(tc.tile_pool(name="aopairp", bufs=3))
        yp = ctx.enter_context(tc.tile_pool(name="youtp", bufs=2))

        # Two filler queues: proj units MUST all be emitted before the pair
        # whose scores read their qkT output (a read emitted before its
        # writer gets no dependency edge -> stale data on HW), so they are
        # high-priority and force-drained at each pair boundary.
        projq = deque()
        outq = deque()

        def fill(n=1):
            c = 0
            while c < n and (projq or outq):
                (projq or outq).popleft()()
                c += 1

        def drain_projq():
            while projq:
                projq.popleft()()

        # ---- startup loads, spread across DMA queues, critical-first ----
        # First QK chains need wq (scalar head-of-queue) + xT0 (sync, split
        # in dt-halves so dt 0..3 matmuls start after half the bytes).
        def emit_proj_load(j, eng=None):
            wqk = wqkp.tile([128, DT, 256], BF16, tag="wqk")
            e = eng or nc.sync
            if eng is None:
                e.dma_start(wqk[:, :, 0:128], wq_r[j])
                e.dma_start(wqk[:, :, 128:256], wk_r[j])
            else:
                # startup: dt-sliced so the first chain's matmuls begin
                # after a fraction of the bytes; wk goes on the OTHER
                # (sync) queue ordered to land just before chain (fs=1)
                e.dma_start(wqk[:, 0:2, 0:128], wq_r[j][:, 0:2, :])
                e.dma_start(wqk[:, 2:8, 0:128], wq_r[j][:, 2:8, :])
                e.dma_start(wqk[:, :, 128:256], wk_r[j])
            return wqk

        wqk0 = emit_proj_load(0, eng=nc.scalar)

        # xTc[c][p, dt, t'] = x[c*512+t', dt*128+p]
        # HWDGE queues are FIFO per issuing engine, so queue order IS
        # priority: first QK chains need wq (scalar head) + xT0 (sync
        # head); everything bulky rides behind them.
        xTc = [xtp.tile([128, DT, 512], BF16, tag=f"xt{c}", name=f"xt{c}")
               for c in range(4)]
        nc.sync.dma_start(xTc[0][:, 0:2, :], xt_r[0, :, 0:2, :])
        nc.sync.dma_start(xTc[0][:, 2:8, :], xt_r[0, :, 2:8, :])
        nc.sync.dma_start(xTc[1][:, :, :], xt_r[1])
        nc.sync.dma_start(xTc[2][:, :, :], xt_r[2])
        # xT3/wv ride the scalar queue BEHIND a gate DMA that waits for
        # xT1's arrival: HWDGE is FIFO per queue, so these bulk loads
        # (needed ~15us later) stop stealing DMA bandwidth from the
        # critical startup loads. (The gate must read a tile whose
        # write is already emitted -- emission order IS the dep graph.)
        gate = xtp.tile([1, 2], BF16, tag="gate")
        nc.scalar.dma_start(gate[:, :], xTc[1][0:1, 7, 510:512])
        wv_t = qkp.tile([128, DT, FH], BF16, tag="wv", bufs=1)
        nc.scalar.dma_start(xTc[3][:, :, :], xt_r[3])
        nc.scalar.dma_start(wv_t[:, :, :], wv_r[:, :, :])

        def load_wout(j):
            w = wop.tile([128, 1, D], BF16, tag="wout")
            nc.scalar.dma_start(w[:, 0, :], wout_r[j])
            return w

        wout_t = load_wout(0)

        # PE warmup: junk matmuls on a memset tile while the first loads
        # are in flight -- the PE pstate ramp (0.65/1.2 GHz for ~3us of
        # continuous execution) completes before real chains start.
        wu = xtp.tile([128, 512], BF16, tag="wu")
        nc.vector.memset(wu[:, :].bitcast(mybir.dt.uint16), 0)
        wups = pcp.tile([128, 512], F32, tag="pc", name="wups")
        for i in range(WARMUP):
            nc.tensor.matmul(wups[:, :], wu[:, 0:128], wu[:, :],
                             start=(i == 0), stop=(i == WARMUP - 1))


        # ---- QK projection: chains of 8 accumulating matmuls + a copy ----
        def proj_chain_units(wqk, qkT_t, fs, tck):
            units = []
            hold = {}

            def mk_mm(dt):
                def u():
                    if dt == 0:
                        hold["psq"] = pcp.tile([128, 512], F32, tag="pc", name="psq")
                    nc.tensor.matmul(
                        hold["psq"][:, :],
                        wqk[:, dt, fs * 128:(fs + 1) * 128],
                        xTc[tck][:, dt, :],
                        start=(dt == 0), stop=(dt == DT - 1))
                return u

            for dt in range(DT):
                units.append(mk_mm(dt))

            def cp():
                nc.vector.tensor_copy(
                    qkT_t[:, fs, tck * 512:(tck + 1) * 512], hold["psq"][:, :])

            units.append(cp)
            return units

        # pair 0 QK projection runs upfront (nothing else to overlap)
        qkT = qkp.tile([128, 2, S], BF16, tag="qkT")
        for fs, tck in PROJ_ORDER:
            for u in proj_chain_units(wqk0, qkT, fs, tck):
                u()

        # ---- V projection (all heads), per k-tile, ones column appended ----
        v1k = [None] * TT

        def emit_vproj(kt):
            if v1k[kt] is not None:
                return
            v1 = v1p.tile([128, HPC, HD + 1], BF16, tag=f"v1_{kt}")
            nc.vector.memset(v1[:, :, HD:HD + 1].bitcast(mybir.dt.uint16),
                             0x3F80)  # bf16 1.0
            xc, sub = xTc[kt // 4], kt % 4
            psv = pcp.tile([128, 512], F32, tag="pc")
            for dt in range(DT):
                nc.tensor.matmul(psv[:, :],
                                 xc[:, dt, sub * 128:(sub + 1) * 128],
                                 wv_t[:, dt, :],
                                 start=(dt == 0), stop=(dt == DT - 1))
            nc.vector.tensor_copy(
                v1[:, :, 0:HD],
                psv[:, :].rearrange("p (h c) -> p h c", c=HD))
            v1k[kt] = v1

        def emit_normalize(hs, q0, pu, ao_pair, last=False):
            # r = 1/rowsum, broadcast across partitions on GpSimd (the DRAM
            # bounce this replaces cost ~3us of latency per chunk).
            # Mid-stream, both PSUM copies go first so pu is freed for the
            # next chunk's attnV asap; for the final chunk latency to the
            # drain matters instead, so the reciprocal chain starts early.
            u = unp.tile([HD + 1, 1024], F32, tag="u")
            r0 = rbp.tile([1, 1024], F32, tag="r0")
            rb = rbp.tile([HD, 1024], F32, tag="rb")
            ao1 = (ao1p.tile([HD, 1024], BF16, tag="ao1", name="ao1")
                   if hs == 1 else None)

            def norm_half(c0, c1):
                # reciprocal reads the rowsum row straight from PSUM (a
                # [1,1024] copy would cost a full 1.1us: one DVE lane)
                nc.vector.reciprocal(r0[:, c0:c1], pu[HD:HD + 1, c0:c1])
                nc.gpsimd.partition_broadcast(rb[:, c0:c1], r0[:, c0:c1])
                nc.vector.tensor_copy(u[0:HD, c0:c1], pu[0:HD, c0:c1])
                if hs == 0:
                    nc.vector.tensor_mul(ao_pair[0:HD, q0 + c0:q0 + c1],
                                         u[0:HD, c0:c1], rb[:, c0:c1])
                else:
                    # head 1 belongs on partitions 64:128: SBUF->SBUF DMA
                    nc.vector.tensor_mul(ao1[:, c0:c1], u[0:HD, c0:c1],
                                         rb[:, c0:c1])
                    nc.sync.dma_start(
                        ao_pair[HD:2 * HD, q0 + c0:q0 + c1],
                        ao1[:, c0:c1])

            if last:
                # halves pipeline with the out-projection drain
                norm_half(0, 512)
                norm_half(512, 1024)
            else:
                norm_half(0, 1024)

        # ---- out-projection: (tt, ec) units ----
        # y accumulates ON-CHIP across the 4 pairs in 16 persistent ysb
        # tiles (64KB/partition); DRAM sees one plain 8MB write at the last
        # pair instead of 32MB of DMA-accumulate (which saturated the DMA
        # engines and stalled the filler pipeline).
        ysb_t = [None] * TT

        def outproj_units(j, qh, ao_pair, wout_t):
            units = []
            for i in range(8):
                if j == NPAIR - 1 and qh == 1:
                    # drain: in-order so the first units depend only on
                    # the last chunk's FIRST normalize half (q columns
                    # complete in order) -- y DMAs start sooner
                    tt = qh * 8 + i
                else:
                    tt = qh * 8 + (i + j * 2) % 8  # rotate per pair

                def mk(tt=tt):
                    if ysb_t[tt] is None:
                        ysb_t[tt] = yp.tile([128, D], F32, tag=f"y{tt}",
                                            name=f"ysb{tt}", bufs=1)
                    ysb = ysb_t[tt]

                    if j == NPAIR - 1 and qh == 1:
                        # drain set (runs after all attention): one wide
                        # unit per tile from the idle scores-PSUM ring;
                        # the tail is bounded by the serial y-DMA chain,
                        # so keep the data path simple
                        def wide():
                            psy = psp.tile([128, 1024], F32, tag="ps",
                                           name="psyw")
                            for e in range(2):
                                nc.tensor.matmul(
                                    psy[:, e * 512:(e + 1) * 512],
                                    ao_pair[:, tt * 128:(tt + 1) * 128],
                                    wout_t[:, 0, e * 512:(e + 1) * 512],
                                    start=True, stop=True)
                            nc.vector.tensor_add(ysb[:, :], ysb[:, :],
                                                 psy[:, :])
                            eng = nc.sync if tt % 2 == 0 else nc.scalar
                            eng.dma_start(
                                y_ap[tt * 128:(tt + 1) * 128, :], ysb[:, :])
                        return [wide]

                    def ec(e):
                        def f():
                            psy = pcp.tile([128, 512], F32, tag="pc",
                                           name="psy")
                            nc.tensor.matmul(
                                psy[:, :],
                                ao_pair[:, tt * 128:(tt + 1) * 128],
                                wout_t[:, 0, e * 512:(e + 1) * 512],
                                start=True, stop=True)
                            dst = ysb[:, e * 512:(e + 1) * 512]
                            if j == 0:
                                nc.vector.tensor_copy(dst, psy[:, :])
                            else:
                                nc.vector.tensor_add(dst, dst, psy[:, :])
                            if j == NPAIR - 1:
                                # plain writes ride the two HWDGE queues,
                                # keeping the Pool engine clear at the tail
                                eng = nc.sync if tt % 2 == 0 else nc.scalar
                                eng.dma_start(
                                    y_ap[tt * 128:(tt + 1) * 128,
                                         e * 512:(e + 1) * 512],
                                    dst)
                        return f

                    return [ec(0), ec(1)]

                units.extend(mk())
            return units

        # ---- one attention chunk: (pair j, head hs, q-half qh) ----
        def chunk(j, hs, qh, qkT, ao_pair, pre=None):
            h8 = j * 2 + hs
            lo, hi = hs * HD, (hs + 1) * HD
            q0 = qh * 1024
            pu = pup.tile([HD + 1, 1024], F32, tag="pu")
            ats = {}

            def scores(kt):
                ps = psp.tile([128, 1024], F32, tag="ps")
                for qc in range(2):
                    nc.tensor.matmul(
                        ps[:, qc * 512:(qc + 1) * 512],
                        qkT[lo:hi, 1, kt * 128:(kt + 1) * 128],
                        qkT[lo:hi, 0, q0 + qc * 512:q0 + (qc + 1) * 512],
                        start=True, stop=True)
                at = attnp.tile([128, 1024], BF16, tag="attn")
                nc.scalar.activation(
                    at[:, :], ps[:, :],
                    func=mybir.ActivationFunctionType.Exp,
                    scale=0.125)
                ats[kt] = at

            def attnv(kt):
                at = ats.pop(kt)
                for qc in range(2):
                    nc.tensor.matmul(
                        pu[0:HD + 1, qc * 512:(qc + 1) * 512],
                        v1k[kt][:, h8, :],
                        at[:, qc * 512:(qc + 1) * 512],
                        start=(kt == 0), stop=(kt == TT - 1))

            if pre:
                pre(0)
            scores(0)
            credit = 0.0
            for kt in range(TT):
                if pre and kt + 1 < TT:
                    pre(kt + 1)
                if kt + 1 < TT:
                    scores(kt + 1)
                # ScalarE paces attention at ~1123ns/kt vs 852ns of PE work:
                # ~1.27 filler matmuls per k-tile closes the gap without
                # draining the deque ahead of later (filler-starved) chunks
                credit += 1.0 if pre else RATE[j]
                take = int(credit)
                fill(1)
                attnv(kt)
                if take > 1:
                    fill(take - 1)
                credit -= take
            emit_normalize(hs, q0, pu, ao_pair,
                           last=(j == NPAIR - 1 and hs == 1 and qh == 1))

        # ---- main pair loop ----
        for j in range(NPAIR):
            ao_pair = aopp.tile([128, S], BF16, tag="aopair")
            if j + 1 < NPAIR:
                wqk_next = emit_proj_load(j + 1)
                wout_next = load_wout(j + 1)
                qkT_next = qkp.tile([128, 2, S], BF16, tag="qkT")
            else:
                wqk_next = wout_next = qkT_next = None

            # pair-0 delays next-pair proj fillers one chunk: qkT(1) writes
            # into the slot ring while wv/vproj still reads its slot mate
            if qkT_next is not None and j > 0:
                for fs, tck in PROJ_ORDER:
                    projq.extend(
                        proj_chain_units(wqk_next, qkT_next, fs, tck))
            chunk(j, 0, 0, qkT, ao_pair,
                  pre=(emit_vproj if j == 0 else None))
            if qkT_next is not None and j == 0:
                for fs, tck in PROJ_ORDER:
                    projq.extend(
                        proj_chain_units(wqk_next, qkT_next, fs, tck))
            chunk(j, 1, 0, qkT, ao_pair)
            # pair 2's outproj is deferred into pair 3, which otherwise has
            # no proj fillers and runs ScalarE-paced (ao ring is 3-deep so
            # pair 3's ao tile doesn't wait on pair 1's readers)
            if j == 2:
                held = outproj_units(j, 0, ao_pair, wout_t)
            else:
                outq.extend(outproj_units(j, 0, ao_pair, wout_t))
            chunk(j, 0, 1, qkT, ao_pair)
            chunk(j, 1, 1, qkT, ao_pair)
            if j == 2:
                held += outproj_units(j, 1, ao_pair, wout_t)
            else:
                outq.extend(outproj_units(j, 1, ao_pair, wout_t))
            drain_projq()
            if j == 2:
                outq.extend(held)

            qkT, wout_t = qkT_next, wout_next

        while outq:
            fill(4)

    nc.compile()
    return nc


_NC = None


def get_nc():
    global _NC
    if _NC is None:
        _NC = build_program()
    return _NC


def _bf16(a):
    return a.astype(mybir.dt.np(BF16))


def _pack_w(w):
    # [D, FH] -> [NPAIR, 128, DT, 128]: pair-major contiguous tile blocks
    return np.ascontiguousarray(
        w.reshape(DT, 128, NPAIR, 128).transpose(2, 1, 0, 3))


def make_in_maps(x, Wqkv, Wout):
    x = np.asarray(x, dtype=np.float32)
    Wqkv = np.asarray(Wqkv, dtype=np.float32)
    Wout = np.asarray(Wout, dtype=np.float32)
    in_maps = []
    for b in range(B):
        # [4, 128, DT, 512]: xt_packed[c, p, dt, t] = x[b][c*512+t, dt*128+p]
        xp = _bf16(np.ascontiguousarray(
            x[b].T.reshape(DT, 128, 4, 512).transpose(2, 1, 0, 3)))
        for hh in range(2):
            c0 = hh * FH
            wv = Wqkv[:, 2 * D + c0:2 * D + c0 + FH]
            in_maps.append({
                "xt": xp,
                "wq": _bf16(_pack_w(Wqkv[:, c0:c0 + FH])),
                "wk": _bf16(_pack_w(Wqkv[:, D + c0:D + c0 + FH])),
                "wv": _bf16(np.ascontiguousarray(
                    wv.reshape(DT, 128, FH).transpose(1, 0, 2))),
                "wout": _bf16(np.ascontiguousarray(
                    Wout[c0:c0 + FH, :].reshape(NPAIR, 128, D))),
            })
    return in_maps


def assemble(results):
    y = np.empty((B, S, D), dtype=np.float32)
    for b in range(B):
        y[b] = results[2 * b]["y"] + results[2 * b + 1]["y"]
    return y


def kernel(x, attn_mask, Wqkv, bqkv, Wout, bout):
    for name, t in (("attn_mask", attn_mask), ("bqkv", bqkv), ("bout", bout)):
        if np.any(np.asarray(t)):
            raise NotImplementedError(f"kernel assumes {name} == 0")
    nc = get_nc()
    res = run_bass_kernel_spmd(nc, make_in_maps(x, Wqkv, Wout),
                               core_ids=list(range(N_CORES)))
    return assemble(res.results)


if __name__ == "__main__":
    rng = np.random.default_rng(0)
    x = rng.standard_normal((B, S, D), dtype=np.float32)
    Wqkv = (rng.standard_normal((D, 3 * D), dtype=np.float32) / np.sqrt(D)).astype(np.float32)
    Wout = (rng.standard_normal((D, D), dtype=np.float32) / np.sqrt(D)).astype(np.float32)
    zeros = np.zeros
    y = kernel(x, zeros((S, S), np.float32), Wqkv, zeros(3 * D, np.float32),
               Wout, zeros(D, np.float32))
    print("y", y.shape, y.dtype, float(np.abs(y).mean()))
